# revision 26
# baseline (speedup 1.0000x reference)
"""Ewald potential Bass kernels for TRN2 (8-core SPMD), v2.

K1 shards k-space (480 cols/core of padded 3840) over all 8192 atoms ->
k_pot re/im (fp32) and v_pot re/im (fp16). Host gathers, computes
akp=|k_pot| and fp16 splits. K2 shards atoms (1024/core): aw GEMM (3-term
fp16 split) -> softmax -> inverse transform via PE-transposed sm.

Phases come from a one-hot selection GEMM against host-precomputed
frac(coord*k) tables centered in [-0.5,0.5]: phase' = Tx+Ty+Tz in
[-1.5,1.5], range-reduced with a single ADD_RANGE_WRAP, cos via a second
wrap (+0.25). Sin activation with scale=2pi.

out[n,d] = sum_k sm[n,k] * (cos(2pi phi_i)*vpr[k,d] + sin(2pi phi_i)*vpi[k,d]) / Z[n]
with eik_i = exp(-2pi i phi_i) = cos - i sin.
"""
import sys
sys.path.insert(0, '/opt/trn_rl_repo')
import numpy as np
import ml_dtypes
import concourse.bass as bass
import concourse.tile as tile
import concourse.mybir as mybir
from concourse import bacc
from concourse.bass_utils import run_bass_kernel_spmd
from contextlib import ExitStack

F = mybir.ActivationFunctionType
DT = mybir.dt
ALU = mybir.AluOpType
AX = mybir.AxisListType

P = 128
N = 8192
D = 128
NK = 12              # grid: kx in [0,12], ky/kz in [-12,12]
KPAD = 3840          # 3796 padded to 30*128
KSH = KPAD // 8      # 480 k-cols per core in K1
NSH = N // 8         # 1024 atoms per core in K2
NCH = N // P         # 64 atom chunks in K1
KCH = KPAD // P      # 30 k chunks in K2
AWK = 4096           # aw/sm width per n-chunk (2 halves of 2048)
NC2 = NSH // P       # 8 atom chunks in K2
NROW = 63            # 13 x-rows + 25 y-rows + 25 z-rows
TWOPI = float(2 * np.pi)

bf16 = ml_dtypes.bfloat16
f16 = np.float16


def _frac_tables(rfrac):
    """[63, n] fp64 tables: frac(coord*u) centered to [-0.5, 0.5]."""
    n = rfrac.shape[0]
    t = np.zeros((NROW, n), dtype=np.float64)
    r64 = rfrac.astype(np.float64)
    for u in range(NK + 1):                      # x rows: u = 0..12
        v = r64[:, 0] * u
        t[u] = v - np.round(v)
    for i, u in enumerate(range(-NK, NK + 1)):   # y rows
        v = r64[:, 1] * u
        t[13 + i] = v - np.round(v)
    for i, u in enumerate(range(-NK, NK + 1)):   # z rows
        v = r64[:, 2] * u
        t[38 + i] = v - np.round(v)
    return t


def _select_mat(kmat):
    """[63, KPAD] fp16 one-hot selection for k rows (padded cols zero)."""
    K = kmat.shape[0]
    s = np.zeros((NROW, KPAD), dtype=np.float32)
    j = np.arange(K)
    s[kmat[:, 0], j] = 1.0
    s[13 + kmat[:, 1] + NK, j] = 1.0
    s[38 + kmat[:, 2] + NK, j] = 1.0
    return s.astype(f16)


def split16(x):
    """fp16 2-way split: x ~ hi + lo to ~2^-22 rel."""
    x = np.asarray(x, dtype=np.float32)
    hi = x.astype(f16)
    lo = (x - hi.astype(np.float32)).astype(f16)
    return hi, lo


def host_prep(q_vector, k_vector, v_vector, positions, cell, k_fwd, k_inv):
    L = float(np.asarray(cell).reshape(3, 3)[0, 0])
    rfrac = np.asarray(positions, dtype=np.float64) / L
    t64 = _frac_tables(rfrac)                     # [63, N]
    th = t64.astype(f16)
    tl = (t64 - th.astype(np.float64)).astype(f16)
    sf = _select_mat(np.asarray(k_fwd))           # [63, KPAD]
    si = _select_mat(np.asarray(k_inv))
    kvh, kvl = split16(k_vector)                  # [N, D]
    vvh = np.asarray(v_vector, dtype=np.float32).astype(f16)
    qh, ql = split16(np.abs(np.asarray(q_vector, dtype=np.float32)).T)  # [D, N]
    return th, tl, sf, si, kvh, kvl, vvh, qh, ql


def chunk_major(x):
    """[N, D] -> [P, NCH*D]: partition=n%P? No: chunk c rows c*P..(c+1)*P
    land at [:, c*D:(c+1)*D]."""
    n, d = x.shape
    c = n // P
    return np.ascontiguousarray(
        x.reshape(c, P, d).transpose(1, 0, 2).reshape(P, c * d))


# ---------------------------------------------------------------- kernel 1
def build_k1():
    nc = bacc.Bacc("TRN2", target_bir_lowering=False, debug=False)
    th_d = nc.dram_tensor("th", [2 * NROW, N], DT.float16, kind="ExternalInput").ap()
    sf_d = nc.dram_tensor("sf", [2 * NROW, KSH], DT.float16, kind="ExternalInput").ap()
    kvh_d = nc.dram_tensor("kvh", [P, NCH * D], DT.float16, kind="ExternalInput").ap()
    vvh_d = nc.dram_tensor("vvh", [P, NCH * D], DT.float16, kind="ExternalInput").ap()
    kre_d = nc.dram_tensor("kre", [P, KSH], DT.float32, kind="ExternalOutput").ap()
    kim_d = nc.dram_tensor("kim", [P, KSH], DT.float32, kind="ExternalOutput").ap()
    vre_d = nc.dram_tensor("vre", [P, KSH], DT.float16, kind="ExternalOutput").ap()
    vim_d = nc.dram_tensor("vim", [P, KSH], DT.float16, kind="ExternalOutput").ap()
    MAGIC = 12582912.0
    HALFPI = float(np.pi / 2)

    with ExitStack() as ctx:
        tc = ctx.enter_context(tile.TileContext(nc))
        cpool = ctx.enter_context(tc.tile_pool(name="const", bufs=1))
        wpool = ctx.enter_context(tc.tile_pool(name="work", bufs=3))
        ppool = ctx.enter_context(tc.tile_pool(name="ptmp", bufs=1))
        php = ctx.enter_context(tc.tile_pool(name="ph", bufs=4, space="PSUM"))
        accp = ctx.enter_context(tc.tile_pool(name="acc", bufs=1, space="PSUM"))

        th = cpool.tile([2 * NROW, N], DT.float16)
        sf = cpool.tile([2 * NROW, KSH], DT.float16)
        kvh = cpool.tile([P, NCH * D], DT.float16)
        vvh = cpool.tile([P, NCH * D], DT.float16)
        NS = N // 8
        DS = NCH * D // 8
        nc.sync.dma_start(sf[:], sf_d)
        for s in range(8):
            nc.sync.dma_start(th[:, s * NS:(s + 1) * NS],
                              th_d[:, s * NS:(s + 1) * NS])
            nc.sync.dma_start(kvh[:, s * DS:(s + 1) * DS],
                              kvh_d[:, s * DS:(s + 1) * DS])
            nc.sync.dma_start(vvh[:, s * DS:(s + 1) * DS],
                              vvh_d[:, s * DS:(s + 1) * DS])

        kre = accp.tile([P, 512], DT.float32)
        kim = accp.tile([P, 512], DT.float32)
        vre = accp.tile([P, 512], DT.float32)
        vim = accp.tile([P, 512], DT.float32)

        halfpi = cpool.tile([P, 1], DT.float32)
        nc.gpsimd.memset(halfpi[:], HALFPI)

        phs = {}

        def emit_ph(c):
            if c >= NCH:
                return
            t = php.tile([P, 512], DT.float32, tag="ph")
            nc.tensor.matmul(t[:, :KSH], th[:, c * P:(c + 1) * P], sf[:],
                             start=True, stop=True)
            phs[c] = t

        def emit_acc(q, sinf, cosf):
            for i in range(4):
                c = 4 * q + i
                sl = slice(i * 512, i * 512 + KSH)
                dsl = slice(c * D, (c + 1) * D)
                nc.tensor.matmul(kre[:, :KSH], kvh[:, dsl], cosf[:, sl],
                                 start=(c == 0), stop=(c == NCH - 1))
                nc.tensor.matmul(vre[:, :KSH], vvh[:, dsl], cosf[:, sl],
                                 start=(c == 0), stop=(c == NCH - 1))
            for i in range(4):
                c = 4 * q + i
                sl = slice(i * 512, i * 512 + KSH)
                dsl = slice(c * D, (c + 1) * D)
                nc.tensor.matmul(kim[:, :KSH], kvh[:, dsl], sinf[:, sl],
                                 start=(c == 0), stop=(c == NCH - 1))
                nc.tensor.matmul(vim[:, :KSH], vvh[:, dsl], sinf[:, sl],
                                 start=(c == 0), stop=(c == NCH - 1))

        NQ = NCH // 4
        for c in range(4):
            emit_ph(c)
        pipe = []
        for q in range(NQ):
            r = wpool.tile([P, 2048], DT.float32, tag="r")
            for i in range(4):
                c = 4 * q + i
                nc.vector.add_range_wrap(r[:, i * 512:(i + 1) * 512],
                                         phs[c][:], 0.0, 0.5, 1.0)
                del phs[c]
            emit_ph(4 * q + 4)
            emit_ph(4 * q + 5)
            sinf = wpool.tile([P, 2048], DT.float16, tag="sinf")
            cosf = wpool.tile([P, 2048], DT.float16, tag="cosf")
            w2 = wpool.tile([P, 2048], DT.float32, tag="w2")
            nc.vector.add_range_wrap(w2[:], r[:], 0.25, 0.5, 1.0)
            nc.scalar.activation(cosf[:], w2[:], F.Sin, scale=TWOPI)
            nc.scalar.activation(sinf[:], r[:], F.Sin, scale=TWOPI)
            emit_ph(4 * q + 6)
            emit_ph(4 * q + 7)
            pipe.append((q, sinf, cosf))
            if len(pipe) > 2:
                emit_acc(*pipe.pop(0))
        for item in pipe:
            emit_acc(*item)

        kre_s = wpool.tile([P, KSH], DT.float32, tag="kre_s")
        kim_s = wpool.tile([P, KSH], DT.float32, tag="kim_s")
        vre_s = wpool.tile([P, KSH], DT.float16, tag="vre_s")
        vim_s = wpool.tile([P, KSH], DT.float16, tag="vim_s")
        nc.vector.tensor_copy(kre_s[:], kre[:, :KSH])
        nc.vector.tensor_copy(kim_s[:], kim[:, :KSH])
        nc.vector.tensor_copy(vre_s[:], vre[:, :KSH])
        nc.vector.tensor_copy(vim_s[:], vim[:, :KSH])
        nc.sync.dma_start(kre_d, kre_s[:])
        nc.sync.dma_start(kim_d, kim_s[:])
        nc.sync.dma_start(vre_d, vre_s[:])
        nc.sync.dma_start(vim_d, vim_s[:])

    nc.compile()
    return nc


# ---------------------------------------------------------------- kernel 2
def build_k2():
    nc = bacc.Bacc("TRN2", target_bir_lowering=False, debug=False)
    t2_d = nc.dram_tensor("t2", [NROW, NSH], DT.float16, kind="ExternalInput").ap()
    si_d = nc.dram_tensor("si", [NROW, KPAD], DT.float16, kind="ExternalInput").ap()
    qh_d = nc.dram_tensor("qh", [P, NSH], DT.float16, kind="ExternalInput").ap()
    ah_d = nc.dram_tensor("ah", [P, AWK], DT.float16, kind="ExternalInput").ap()
    al_d = nc.dram_tensor("al", [P, AWK], DT.float16, kind="ExternalInput").ap()
    vprT_d = nc.dram_tensor("vprT", [P, KCH * D], DT.bfloat16,
                            kind="ExternalInput").ap()
    vpiT_d = nc.dram_tensor("vpiT", [P, KCH * D], DT.bfloat16,
                            kind="ExternalInput").ap()
    ident_d = nc.dram_tensor("ident", [P, P], DT.bfloat16, kind="ExternalInput").ap()
    outT0_d = nc.dram_tensor("outT0", [P, NSH], DT.float32, kind="ExternalOutput").ap()
    outT1_d = nc.dram_tensor("outT1", [P, NSH], DT.float32, kind="ExternalOutput").ap()
    zs_d = nc.dram_tensor("zs", [P, 4 * NC2], DT.float32, kind="ExternalOutput").ap()
    mxs_d = nc.dram_tensor("mxs", [P, 2 * NC2], DT.float32, kind="ExternalOutput").ap()

    with ExitStack() as ctx:
        tc = ctx.enter_context(tile.TileContext(nc))
        cpool = ctx.enter_context(tc.tile_pool(name="const", bufs=1))
        smpool = ctx.enter_context(tc.tile_pool(name="sm", bufs=1))
        wpool = ctx.enter_context(tc.tile_pool(name="work", bufs=3))
        tpool = ctx.enter_context(tc.tile_pool(name="trig", bufs=2))
        zpool = ctx.enter_context(tc.tile_pool(name="z", bufs=1))

        t2 = cpool.tile([NROW, NSH], DT.float16)
        si = cpool.tile([NROW, KPAD], DT.float16)
        qh = cpool.tile([P, NSH], DT.float16)
        ah = cpool.tile([P, AWK], DT.float16)
        al = cpool.tile([P, AWK], DT.float16)
        vprT = cpool.tile([P, KCH * D], DT.bfloat16)
        vpiT = cpool.tile([P, KCH * D], DT.bfloat16)
        ident = cpool.tile([P, P], DT.bfloat16)
        nc.sync.dma_start(qh[:], qh_d)
        for s in range(4):
            ssl = slice(s * 1024, (s + 1) * 1024)
            nc.sync.dma_start(ah[:, ssl], ah_d[:, ssl])
            nc.sync.dma_start(al[:, ssl], al_d[:, ssl])
        nc.sync.dma_start(si[:], si_d)
        nc.sync.dma_start(t2[:], t2_d)
        nc.sync.dma_start(vprT[:], vprT_d)
        nc.sync.dma_start(vpiT[:], vpiT_d)
        nc.sync.dma_start(ident[:], ident_d)

        sm = smpool.tile([P, NC2 * 4096], DT.bfloat16)
        zacc = zpool.tile([P, 4 * NC2], DT.float32)
        mxacc = zpool.tile([P, 2 * NC2], DT.float32)

        # ---- pass 1: aw (2-term fp16) -> per-half softmax (host rescales).
        # Each half = two [128,1024] PSUM quarter-tiles (bufs=2 -> ping-pong);
        # rmax per quarter overlaps the next quarter's GEMMs.
        with tc.tile_pool(name="awps", bufs=2, space="PSUM") as awps:
            for c8 in range(NC2):
                nsl = slice(c8 * P, (c8 + 1) * P)
                for h in range(2):
                    i = 2 * c8 + h
                    qt = []
                    mqs = []
                    for qq in range(2):
                        aw = awps.tile([P, 1024], DT.float32, tag=f"aw{qq}")
                        for j in range(2):
                            ksl = slice(h * 2048 + qq * 1024 + j * 512,
                                        h * 2048 + qq * 1024 + (j + 1) * 512)
                            osl = slice(j * 512, (j + 1) * 512)
                            nc.tensor.matmul(aw[:, osl], qh[:, nsl],
                                             ah[:, ksl], start=True, stop=False)
                            nc.tensor.matmul(aw[:, osl], qh[:, nsl],
                                             al[:, ksl], start=False, stop=True)
                        mq = wpool.tile([P, 1], DT.float32, tag=f"mq{qq}")
                        nc.vector.reduce_max(mq[:], aw[:], axis=AX.X)
                        qt.append(aw)
                        mqs.append(mq)
                    nc.vector.tensor_tensor(mxacc[:, i:i + 1], mqs[0][:],
                                            mqs[1][:], ALU.max)
                    negmx = wpool.tile([P, 1], DT.float32, tag="negmx")
                    nc.vector.tensor_scalar_mul(negmx[:], mxacc[:, i:i + 1],
                                                -1.0)
                    for qq in range(2):
                        base = c8 * AWK + h * 2048 + qq * 1024
                        nc.scalar.activation(
                            sm[:, base: base + 1024], qt[qq][:], F.Exp,
                            bias=negmx[:],
                            accum_out=zacc[:, 2 * i + qq: 2 * i + qq + 1])

        # ---- pass 2: phases -> sin/cos -> PE-transpose sm (DMA-staged to
        # SBUF) -> smC on DVE / smS on Pool -> inverse GEMM (2 kc behind)
        with (tc.tile_pool(name="php", bufs=1, space="PSUM") as php,
              tc.tile_pool(name="smtp", bufs=2, space="PSUM") as smtp,
              tc.tile_pool(name="ops", bufs=1, space="PSUM") as ops):
            outT0 = ops.tile([P, NSH], DT.float32)
            outT1 = ops.tile([P, NSH], DT.float32)
            phs = {}

            def emit_ph(kc):
                if kc >= KCH:
                    return
                t = php.tile([P, 1024], DT.float32, tag="ph")
                for h in range(2):
                    nc.tensor.matmul(t[:, h * 512:(h + 1) * 512],
                                     si[:, kc * P:(kc + 1) * P],
                                     t2[:, h * 512:(h + 1) * 512],
                                     start=True, stop=True)
                phs[kc] = t

            def emit_inv(kc, smC, smS):
                dsl = slice(kc * D, (kc + 1) * D)
                outT = outT0 if kc < 16 else outT1
                first = kc == 0 or kc == 16
                last = kc == 15 or kc == KCH - 1
                for h in range(2):
                    sl = slice(h * 512, (h + 1) * 512)
                    nc.tensor.matmul(outT[:, sl], vprT[:, dsl], smC[:, sl],
                                     start=first, stop=False)
                    nc.tensor.matmul(outT[:, sl], vpiT[:, dsl], smS[:, sl],
                                     start=False, stop=last)

            emit_ph(0)
            emit_ph(1)
            emit_ph(2)
            emit_ph(3)
            pipe = []
            trig = {}
            for pk in range(KCH // 2):
                a, b = 2 * pk, 2 * pk + 1
                r = tpool.tile([P, 2048], DT.float32, tag="r")
                w2 = tpool.tile([P, 2048], DT.float32, tag="w2")
                for h, kc2 in ((0, a), (1, b)):
                    sl = slice(h * 1024, (h + 1) * 1024)
                    nc.vector.add_range_wrap(r[:, sl], phs[kc2][:], 0.0, 0.5,
                                             1.0)
                    nc.vector.add_range_wrap(w2[:, sl], r[:, sl], 0.25, 0.5,
                                             1.0)
                    del phs[kc2]
                emit_ph(2 * pk + 4)
                emit_ph(2 * pk + 5)
                sintp = tpool.tile([P, 2048], DT.bfloat16, tag="sintp")
                costp = tpool.tile([P, 2048], DT.bfloat16, tag="costp")
                nc.scalar.activation(costp[:], w2[:], F.Sin, scale=TWOPI)
                nc.scalar.activation(sintp[:], r[:], F.Sin, scale=TWOPI)
                trig[a] = (sintp, costp, 0)
                trig[b] = (sintp, costp, 1)
                for kc in (a, b):
                    smT = smtp.tile([P, 1024], DT.bfloat16, tag="smT")
                    for c8 in range(NC2):
                        nc.tensor.transpose(
                            smT[:, c8 * P:(c8 + 1) * P],
                            sm[:, c8 * AWK + kc * P:
                               c8 * AWK + (kc + 1) * P],
                            ident[:])
                    sintp_, costp_, hh = trig[kc]
                    tsl = slice(hh * 1024, (hh + 1) * 1024)
                    smC = wpool.tile([P, 1024], DT.bfloat16, tag="smC")
                    smS = wpool.tile([P, 1024], DT.bfloat16, tag="smS")
                    nc.vector.tensor_mul(smC[:], smT[:], costp_[:, tsl])
                    nc.vector.tensor_mul(smS[:], smT[:], sintp_[:, tsl])
                    pipe.append((kc, smC, smS))
                    if len(pipe) > 2:
                        emit_inv(*pipe.pop(0))
            for item in pipe:
                emit_inv(*item)

            res0 = zpool.tile([P, NSH], DT.float32)
            res1 = zpool.tile([P, NSH], DT.float32)
            nc.vector.tensor_copy(res0[:], outT0[:])
            nc.vector.tensor_copy(res1[:], outT1[:])
            nc.sync.dma_start(outT0_d, res0[:])
            nc.sync.dma_start(outT1_d, res1[:])
            nc.sync.dma_start(zs_d, zacc[:])
            nc.sync.dma_start(mxs_d, mxacc[:])

    nc.compile()
    return nc


# ---------------------------------------------------------------- profiling
def enable_ntff_profiling():
    """Provide the antenv.axon_hooks module run_bass_kernel_spmd needs for
    trace=True under axon, backed by trn_boot's ctypes NTFF hook."""
    import types
    if "antenv.axon_hooks" in sys.modules:
        return True
    sys.path.insert(0, "/root/.axon_site")
    try:
        from trn_agent_boot.trn_boot import _ntff_profile_via_ctypes
        hook = _ntff_profile_via_ctypes("/opt/axon/libaxon_pjrt.so")
    except Exception as e:
        print(f"ntff hook unavailable: {e}")
        return False
    if hook is None:
        print("ntff hook: .so lacks axon_start_nrt_profile")
        return False
    mod = types.ModuleType("antenv.axon_hooks")
    mod._hook = hook
    mod.get_axon_ntff_profile_hook = lambda: mod._hook
    mod.set_axon_ntff_profile_hook = lambda h: setattr(mod, "_hook", h)
    sys.modules["antenv.axon_hooks"] = mod
    # upload_artifacts copies the NEFF dir to a remote bucket -- hangs in
    # this container; keep artifacts local instead.
    import concourse.bass_utils as bu
    bu.upload_artifacts = lambda tmpdir: tmpdir
    return True


# ---------------------------------------------------------------- runner
_NC1 = None
_NC2 = None


def run_ewald(q_vector, k_vector, v_vector, positions, cell, batch, k_fwd,
              k_inv, trace=False):
    global _NC1, _NC2
    if trace:
        trace = enable_ntff_profiling()
    th, tl, sf, si, kvh, kvl, vvh, qh, ql = host_prep(
        q_vector, k_vector, v_vector, positions, cell, k_fwd, k_inv)

    kvh_c = chunk_major(kvh)
    vvh_c = chunk_major(vvh)

    th2 = np.concatenate([th, tl], axis=0)          # [126, N]
    sf2 = np.concatenate([sf, sf], axis=0)          # [126, KPAD]
    if _NC1 is None:
        _NC1 = build_k1()
    in1 = [{"th": th2,
            "sf": np.ascontiguousarray(sf2[:, c * KSH:(c + 1) * KSH]),
            "kvh": kvh_c, "vvh": vvh_c} for c in range(8)]
    r1 = run_bass_kernel_spmd(_NC1, in1, list(range(8)), trace=trace)

    K = k_fwd.shape[0]
    kre = np.concatenate([r1.results[c]["kre"] for c in range(8)], axis=1)
    kim = np.concatenate([r1.results[c]["kim"] for c in range(8)], axis=1)
    vre = np.concatenate(
        [r1.results[c]["vre"].astype(np.float32) for c in range(8)], axis=1)
    vim = np.concatenate(
        [r1.results[c]["vim"].astype(np.float32) for c in range(8)], axis=1)
    akp = np.zeros((D, AWK), dtype=np.float32)
    akp[:, :KPAD] = np.hypot(kre, kim)
    akp[:, K:] = 0.0
    ah, al = split16(akp)
    vprT = chunk_major(np.ascontiguousarray(vre.T).astype(bf16))  # [P, KCH*D]
    vpiT = chunk_major(np.ascontiguousarray(vim.T).astype(bf16))
    ident = np.eye(P, dtype=np.float32).astype(bf16)

    if _NC2 is None:
        _NC2 = build_k2()
    in2 = [{"t2": np.ascontiguousarray(th[:, c * NSH:(c + 1) * NSH]),
            "si": si,
            "qh": np.ascontiguousarray(qh[:, c * NSH:(c + 1) * NSH]),
            "ah": ah, "al": al, "vprT": vprT, "vpiT": vpiT, "ident": ident}
           for c in range(8)]
    r2 = run_bass_kernel_spmd(_NC2, in2, list(range(8)), trace=trace)

    outs = []
    for c in range(8):
        o0 = r2.results[c]["outT0"].T                 # [1024 n, 128 d]
        o1 = r2.results[c]["outT1"].T
        zs = r2.results[c]["zs"]                      # [128, 16]
        mxs = r2.results[c]["mxs"].astype(np.float64)
        z0 = (zs[:, 0::4] + zs[:, 1::4]).T.reshape(-1)
        z1 = (zs[:, 2::4] + zs[:, 3::4]).T.reshape(-1)
        m0 = mxs[:, 0::2].T.reshape(-1); m1 = mxs[:, 1::2].T.reshape(-1)
        mx = np.maximum(m0, m1)
        s0 = np.exp(m0 - mx); s1 = np.exp(m1 - mx)
        z = z0 * s0 + z1 * s1
        o = (o0 * s0[:, None] + o1 * s1[:, None]) / z[:, None]
        outs.append(o.astype(np.float32))
    out = np.concatenate(outs, axis=0)
    return out, (r1, r2)


# ---------------------------------------------------------------- entry point
def kernel(q_vector, k_vector, v_vector, positions, cell, batch, k_fwd, k_inv):
    """Full-input entry: shards across 8 NeuronCores internally."""
    out, _ = run_ewald(np.asarray(q_vector), np.asarray(k_vector),
                       np.asarray(v_vector), np.asarray(positions),
                       np.asarray(cell), np.asarray(batch),
                       np.asarray(k_fwd), np.asarray(k_inv))
    return out
